# revision 1
# baseline (speedup 1.0000x reference)
"""Block-causal attention (BlockDiffusionDecoder) on 8 TRN2 NeuronCores.

Reference computes, per (b, h):
    S = (Q K^T) / 8, masked so query block i (64 rows) attends key blocks <= i,
    O = softmax(S) V,   shapes [2, 16, 2048, 64] f32.

Sharding: batch*heads (32) split across 8 cores, 4 heads per core, no comm.

Per-core algorithm (all matmuls bf16, fp32 accumulate):
  - Q,K are staged to DRAM bf16 [2048, 128] (two heads side by side) and
    transposed via the DMA xbar into SBUF [128, 2048]: partitions 0:64 hold
    head-even Q^T/K^T, 64:128 head-odd (row-tiled matmuls at base 64).
  - Scores are computed transposed (S^T tile [128 k, 256 q]): stationary
    K^T_j [64, 128], moving Q^T [64, 256] (two q-tiles per step so the
    stationary load hides under the 256-cycle moving stream).
  - exp via ScalarE from PSUM in [128, <=1024] batches, no max-subtraction
    (|scores/8| <= ~6 so fp32/bf16 exp is safe), output P^T in SBUF bf16.
  - Block mask applied by zeroing P^T sub-blocks (memset).
  - PV: stationary V'_j [128 k, 65] (V plus a ones column -> row sums land
    in output row 64), moving P^T, accumulating O^T [65, 256] in PSUM.
  - O^T -> O via DMA xbar transpose (bf16), then normalize rows by the
    reciprocal of the softmax sum on VectorE and DMA out as f32.
"""

import numpy as np

B, H, S, D = 2, 16, 2048, 64
N_CORES = 8
HPC = (B * H) // N_CORES  # heads per core = 4
QP = S // 256  # q-pairs (two 128-row q-tiles per step) = 8

_CACHE = {}


def _build():
    import concourse.bass as bass
    import concourse.mybir as mybir
    from concourse import bacc
    from concourse.bass import ts
    from concourse.tile import TileContext

    f32 = mybir.dt.float32
    bf16 = mybir.dt.bfloat16

    nc = bacc.Bacc("TRN2", target_bir_lowering=False, debug=False,
                   num_devices=N_CORES)
    q = nc.declare_dram_parameter("q", [HPC, S, D], f32, isOutput=False)
    k = nc.declare_dram_parameter("k", [HPC, S, D], f32, isOutput=False)
    v = nc.declare_dram_parameter("v", [HPC, S, D], f32, isOutput=False)
    out = nc.declare_dram_parameter("out", [HPC, S, D], f32, isOutput=True)

    def off(j):  # free offset of k-tile j's 256-wide block inside pT
        return (j // 4) * 1024 + (j % 4) * 256

    with TileContext(nc) as tc:
        with (
            tc.tile_pool(name="dram_stage", bufs=2, space="DRAM") as dpool,
            tc.tile_pool(name="qkT", bufs=2) as qkT_pool,
            tc.tile_pool(name="vsb", bufs=2) as v_pool,
            tc.tile_pool(name="pT", bufs=3) as pT_pool,
            tc.tile_pool(name="osb", bufs=2) as o_pool,
            tc.tile_pool(name="ps", bufs=3, space="PSUM") as ps_pool,
            tc.tile_pool(name="oT", bufs=2, space="PSUM") as oT_pool,
        ):
            # ---- per head-pair input staging ----
            qT = {}
            kT = {}
            for p in range(HPC // 2):
                sq = dpool.tile([S, 128], bf16, name=f"sq{p}")
                sk = dpool.tile([S, 128], bf16, name=f"sk{p}")
                for e in range(2):
                    h = 2 * p + e
                    nc.gpsimd.dma_start(out=sq[:, e * 64:(e + 1) * 64], in_=q[h])
                    nc.gpsimd.dma_start(out=sk[:, e * 64:(e + 1) * 64], in_=k[h])
                qT_t = qkT_pool.tile([128, S], bf16, name=f"qT{p}", tag="qT")
                kT_t = qkT_pool.tile([128, S], bf16, name=f"kT{p}", tag="kT")
                nc.sync.dma_start_transpose(out=qT_t[:], in_=sq[:])
                nc.sync.dma_start_transpose(out=kT_t[:], in_=sk[:])
                qT[p], kT[p] = qT_t, kT_t

            # ---- flat work list: one item per (head, q-pair) ----
            v_sb = {}
            oT_sb = {}

            def emit_scores(h, t):
                """matmuls + exp + mask for head h, q-pair t. Returns pT tile."""
                p, hb = h // 2, 64 * (h % 2)
                jmax = 2 * t + 1
                pT = pT_pool.tile([128, 4096], bf16, name=f"pT_{h}_{t}", tag="pT")
                for g in range(0, (jmax + 4) // 4):
                    jn = min(4, jmax + 1 - 4 * g)
                    ps = ps_pool.tile([128, 1024], mybir.dt.float32,
                                      name=f"ps_{h}_{t}_{g}", tag="ps")
                    for jj in range(jn):
                        j = 4 * g + jj
                        nc.tensor.matmul(
                            ps[:, jj * 256:(jj + 1) * 256],
                            kT[p][hb:hb + 64, ts(j, 128)],
                            qT[p][hb:hb + 64, t * 256:(t + 1) * 256],
                            start=True, stop=True,
                        )
                    nc.scalar.activation(
                        pT[:, g * 1024:g * 1024 + jn * 256],
                        ps[:, :jn * 256],
                        mybir.ActivationFunctionType.Exp,
                        scale=0.125,
                    )
                # mask: zero P^T blocks where k-block > q-block
                a = off(2 * t)
                nc.gpsimd.memset(pT[64:128, a:a + 64], 0.0)
                b = off(2 * t + 1)
                nc.gpsimd.memset(pT[:, b:b + 128], 0.0)
                nc.gpsimd.memset(pT[64:128, b + 128:b + 192], 0.0)
                return pT

            def emit_pv(h, t, pT):
                jmax = 2 * t + 1
                oT = oT_pool.tile([65, 256], mybir.dt.float32,
                                  name=f"oT_{h}_{t}", tag="oT")
                for j in range(jmax + 1):
                    nc.tensor.matmul(
                        oT[:], v_sb[h][:, j, :], pT[:, off(j):off(j) + 256],
                        start=(j == 0), stop=(j == jmax),
                    )
                nc.vector.tensor_copy(oT_sb[h][0:65, t * 256:(t + 1) * 256], oT[:])

            def emit_head_pre(h):
                vs = v_pool.tile([128, 16, 65], bf16, name=f"v{h}", tag="v")
                nc.gpsimd.dma_start(
                    out=vs[:, :, 0:64],
                    in_=v[h].rearrange("(n p) d -> p n d", p=128),
                )
                nc.vector.memset(vs[:, :, 64], 1.0)
                v_sb[h] = vs
                oT_sb[h] = o_pool.tile([128, S], bf16, name=f"o{h}", tag="osb")

            def emit_head_post(h):
                ot = o_pool.tile([128, 16, 80], bf16, name=f"ot{h}", tag="ot")
                nc.sync.dma_start_transpose(out=ot[:], in_=oT_sb[h][0:80, :])
                rec = o_pool.tile([128, 16], mybir.dt.float32,
                                  name=f"rec{h}", tag="rec")
                nc.vector.reciprocal(rec[:], ot[:, :, 64])
                of = o_pool.tile([128, 16, 64], mybir.dt.float32,
                                 name=f"of{h}", tag="of")
                for n in range(16):
                    nc.vector.tensor_scalar_mul(
                        of[:, n, :], ot[:, n, 0:64], rec[:, n:n + 1])
                nc.sync.dma_start(
                    out=out[h].rearrange("(n p) d -> p n d", p=128), in_=of[:])

            # software-pipelined issue: scores(t) ahead of pv(t-1)
            items = [(h, t) for h in range(HPC) for t in range(QP)]
            pending = None  # (h, t, pT)
            for h, t in items:
                if t == 0:
                    emit_head_pre(h)
                pT = emit_scores(h, t)
                if pending is not None:
                    ph, pt, ppT = pending
                    emit_pv(ph, pt, ppT)
                    if pt == QP - 1:
                        emit_head_post(ph)
                pending = (h, t, pT)
            ph, pt, ppT = pending
            emit_pv(ph, pt, ppT)
            emit_head_post(ph)

    nc.compile()
    return nc


def _get_nc():
    if "nc" not in _CACHE:
        _CACHE["nc"] = _build()
    return _CACHE["nc"]


def kernel(q, k, v):
    from concourse.bass_utils import run_bass_kernel_spmd

    nc = _get_nc()
    qf = np.ascontiguousarray(q, dtype=np.float32).reshape(B * H, S, D)
    kf = np.ascontiguousarray(k, dtype=np.float32).reshape(B * H, S, D)
    vf = np.ascontiguousarray(v, dtype=np.float32).reshape(B * H, S, D)
    in_maps = [
        {
            "q": qf[c * HPC:(c + 1) * HPC],
            "k": kf[c * HPC:(c + 1) * HPC],
            "v": vf[c * HPC:(c + 1) * HPC],
        }
        for c in range(N_CORES)
    ]
    res = run_bass_kernel_spmd(nc, in_maps, core_ids=list(range(N_CORES)))
    full = np.concatenate([res.results[c]["out"] for c in range(N_CORES)], axis=0)
    return full.reshape(B, H, S, D).astype(np.float32)
